# revision 3
# baseline (speedup 1.0000x reference)
"""Bass/Trainium2 kernel for the Tacotron2-style decoder (nn_Decoder).

Parallelization (8 cores, one chip):
- Both LSTMs: model-parallel over the gate/hidden dim. Core j owns hidden
  units [j*128,(j+1)*128) of each LSTM; computes gates [16, 512] =
  [i|f|g|o] via weights-moving matmuls (stationary = transposed
  activations, moving = transposed weight slices).
- Attention: batch-parallel. Core j owns batches {2j, 2j+1}: conv via
  im2col matmul, energies, softmax, context.
- Per step, two SBUF->SBUF all-gathers via remote_dma_broadcast:
  AG-A: [ah.T(t) | dh.T(t-1)] chunk [128, 32] per core
  AG-B: ctx.T chunk [128, (4,2)] per core
  Each core broadcasts its staged payload into slot `pid` of every
  peer's gather buffer (register-offset out_ap).
- proj runs replicated, one step delayed (needs gathered dh).
"""

import sys

sys.path.insert(0, "/opt/trn_rl_repo")

from contextlib import ExitStack

import numpy as np

import concourse.bass as bass
import concourse.mybir as mybir
import concourse.tile as tile
from concourse import bacc
from concourse.masks import make_identity
from concourse.tile_rust import add_dep_helper

FP32 = mybir.dt.float32
BF16 = mybir.dt.bfloat16
AF = mybir.ActivationFunctionType

B, T_ENC = 16, 200
N_MELS, PRENET, ENC_H = 80, 256, 512
H = 1024  # ATTN_H == DEC_H
D = 128  # ATTN_D
KSIZE = 31
W = 8  # cores
NB = 2  # batches per core
GS = 512  # gate slice per core
HS = 128  # hidden slice per core
TEP = 230  # 15 + 200 + 15
LT = NB * T_ENC  # 400


def _kt(x):  # [K, N] -> [K//128, 128, N]
    K = x.shape[0]
    assert K % 128 == 0, K
    return np.ascontiguousarray(x.reshape(K // 128, 128, x.shape[1]))


def prep_inputs(inputs, t_dec):
    import ml_dtypes

    bf = lambda x: np.ascontiguousarray(x).astype(ml_dtypes.bfloat16)
    f32 = lambda k: np.asarray(inputs[k], np.float32)

    enc = f32("encoder_output")
    targets = f32("targets")
    prenet_w, prenet_b = f32("prenet_w"), f32("prenet_b")
    M_w, Q_w, W_w, L_w = f32("M_w"), f32("Q_w"), f32("W_w"), f32("L_w")
    conv_w = f32("conv_w")
    a_wih, a_whh = f32("alstm_wih"), f32("alstm_whh")
    a_b = f32("alstm_bih") + f32("alstm_bhh")
    d_wih, d_whh = f32("dlstm_wih"), f32("dlstm_whh")
    d_b = f32("dlstm_bih") + f32("dlstm_bhh")
    proj_w, proj_b = f32("proj_w"), f32("proj_b")

    WL = np.einsum("df,fck->ckd", L_w, conv_w).reshape(2 * KSIZE, D)

    dec_in = np.concatenate([np.zeros((B, 1, N_MELS), np.float32), targets], axis=1)[
        :, :t_dec
    ]
    dec_inT = np.ascontiguousarray(dec_in.transpose(2, 1, 0)).reshape(
        N_MELS, t_dec * B
    )

    shared = {
        "qw_t": bf(_kt(Q_w.T)),
        "mw_t": bf(_kt(M_w.T)),
        "ww_t": bf(W_w.T),
        "wl": np.ascontiguousarray(WL, np.float32),
        "proj_t": bf(_kt(proj_w.T)),
        "proj_bias": bf(proj_b[None, :]),
        "prenet_w16": bf(prenet_w.T),
        "prenet_b2": np.ascontiguousarray(prenet_b.reshape(2, 128).T, np.float32),
        "dec_inT": bf(dec_inT),
    }

    in_maps = []
    for j in range(W):
        idx = np.arange(j * HS, (j + 1) * HS)
        rows = np.r_[idx, H + idx, 2 * H + idx, 3 * H + idx]
        wa = a_wih[rows]
        wd = d_wih[rows]
        m = dict(shared)
        m["wih_a_x"] = bf(_kt(wa[:, :PRENET].T))
        m["wih_a_c"] = bf(_kt(wa[:, PRENET:].T))
        m["whh_a"] = bf(_kt(a_whh[rows].T))
        m["wih_d_c"] = bf(_kt(wd[:, :ENC_H].T))
        m["wih_d_h"] = bf(_kt(wd[:, ENC_H:].T))
        m["whh_d"] = bf(_kt(d_whh[rows].T))
        m["bias_a"] = bf(a_b[rows][None, :])
        m["bias_d"] = bf(d_b[rows][None, :])

        bs = [NB * j + lb for lb in range(NB)]
        enc_own = enc[bs]  # (2, 200, 512)
        enc_stat = np.zeros((NB, 2, 128, ENC_H), np.float32)
        enc_stat[:, 0] = enc_own[:, :128]
        enc_stat[:, 1, :72] = enc_own[:, 128:]
        m["enc_stat"] = bf(enc_stat)
        encT = np.ascontiguousarray(enc_own.transpose(2, 0, 1)).reshape(ENC_H, LT)
        m["encT_own"] = bf(encT.reshape(4, 128, LT))
        indm = np.zeros((B, LT), np.float32)
        for lb in range(NB):
            indm[NB * j + lb, lb * T_ENC : (lb + 1) * T_ENC] = 1.0
        m["ind"] = bf(indm)
        in_maps.append(m)
    return in_maps


def assemble_outputs(results, t_dec):
    mel = np.asarray(results[0]["mels_out"])  # [t, 16, 80]
    mel_outputs = np.ascontiguousarray(mel.transpose(1, 0, 2), dtype=np.float32)
    align = np.zeros((B, t_dec, T_ENC), np.float32)
    for j in range(W):
        a = np.asarray(results[j]["aligns_out"])  # [t, 2, 200]
        for lb in range(NB):
            align[NB * j + lb] = a[:, lb]
    return mel_outputs, align


def build_decoder(t_dec):
    nc = bacc.Bacc("TRN2", target_bir_lowering=False, debug=False, num_devices=W)

    dp = lambda n, s, dt: nc.dram_tensor(n, list(s), dt, kind="ExternalInput").ap()
    wih_a_x = dp("wih_a_x", (2, 128, GS), BF16)
    wih_a_c = dp("wih_a_c", (4, 128, GS), BF16)
    whh_a = dp("whh_a", (8, 128, GS), BF16)
    wih_d_c = dp("wih_d_c", (4, 128, GS), BF16)
    wih_d_h = dp("wih_d_h", (8, 128, GS), BF16)
    whh_d = dp("whh_d", (8, 128, GS), BF16)
    bias_a = dp("bias_a", (1, GS), BF16)
    bias_d = dp("bias_d", (1, GS), BF16)
    qw_t = dp("qw_t", (8, 128, D), BF16)
    mw_t = dp("mw_t", (4, 128, D), BF16)
    ww_t = dp("ww_t", (D, 1), BF16)
    wl = dp("wl", (2 * KSIZE, D), FP32)
    proj_t = dp("proj_t", (12, 128, N_MELS), BF16)
    proj_bias = dp("proj_bias", (1, N_MELS), BF16)
    prenet_w16 = dp("prenet_w16", (N_MELS, PRENET), BF16)
    prenet_b2 = dp("prenet_b2", (128, 2), FP32)
    dec_inT = dp("dec_inT", (N_MELS, t_dec * B), BF16)
    enc_stat = dp("enc_stat", (NB, 2, 128, ENC_H), BF16)
    encT_own = dp("encT_own", (4, 128, LT), BF16)
    ind = dp("ind", (B, LT), BF16)

    mels_out = nc.dram_tensor(
        "mels_out", [t_dec, B, N_MELS], FP32, kind="ExternalOutput"
    ).ap()
    aligns_out = nc.dram_tensor(
        "aligns_out", [t_dec, NB, T_ENC], FP32, kind="ExternalOutput"
    ).ap()
    cA_in = nc.dram_tensor("cA_in", [128, 2 * B], BF16).ap()
    cA_out = nc.dram_tensor("cA_out", [W, 128, 2 * B], BF16, addr_space="Shared").ap()
    cB_in = nc.dram_tensor("cB_in", [128, 4 * NB], BF16).ap()
    cB_out = nc.dram_tensor("cB_out", [W, 128, 4 * NB], BF16, addr_space="Shared").ap()

    sbt = lambda n, s, dt: nc.alloc_sbuf_tensor(n, list(s), dt)
    s_wax = sbt("s_wax", (128, 2, GS), BF16)
    s_wac = sbt("s_wac", (128, 4, GS), BF16)
    s_wha = sbt("s_wha", (128, 8, GS), BF16)
    s_wdc = sbt("s_wdc", (128, 4, GS), BF16)
    s_wdh = sbt("s_wdh", (128, 8, GS), BF16)
    s_whd = sbt("s_whd", (128, 8, GS), BF16)
    s_ba = sbt("s_ba", (1, GS), BF16)
    s_bd = sbt("s_bd", (1, GS), BF16)
    s_qw = sbt("s_qw", (128, 8, D), BF16)
    s_mw = sbt("s_mw", (128, 4, D), BF16)
    s_ww = sbt("s_ww", (D, 1), BF16)
    s_wl = sbt("s_wl", (2 * KSIZE, D), FP32)
    s_proj = sbt("s_proj", (128, 12, N_MELS), BF16)
    s_pb = sbt("s_pb", (1, N_MELS), BF16)
    s_prw = sbt("s_prw", (N_MELS, PRENET), BF16)
    s_prb = sbt("s_prb", (128, 2), FP32)
    s_decT = sbt("s_decT", (N_MELS, t_dec * B), BF16)
    s_enc = sbt("s_enc", (128, NB, 2, ENC_H), BF16)  # [p, lb, tt, e]
    s_encT = sbt("s_encT", (128, 4, LT), BF16)
    s_ind = sbt("s_ind", (B, LT), BF16)

    s_xsT = sbt("s_xsT", (128, 2, t_dec * B), BF16)
    s_pme = sbt("s_pme", (128, LT), FP32)
    s_bufA = sbt("s_bufA", (128, 3, W, 2 * B), BF16)  # [p, par, src, 32]
    s_bufB = sbt("s_bufB", (128, 3, 4, W, NB), BF16)  # [p, par, et, src, lb]
    s_payA = sbt("s_payA", (128, 2, 2 * B), BF16)  # [p, t%2, 32]
    s_payB = sbt("s_payB", (128, 2, 4 * NB), BF16)  # [p, t%2, (et,lb)]
    s_wbuf = sbt("s_wbuf", (1, 2, NB, TEP), FP32)  # [., c, lb, 230]
    s_ca = sbt("s_ca", (B, HS), FP32)
    s_cd = sbt("s_cd", (B, HS), FP32)
    s_id = sbt("s_id", (16, 16), FP32)
    s_ones = sbt("s_ones", (1, B), BF16)
    s_wT = sbt("s_wT", (128, 2, NB, 2), BF16)  # [tp, t%2, lb, tt]
    s_q16 = sbt("s_q16", (B, 2, D), BF16)
    s_th = sbt("s_th", (128, 2, LT), BF16)
    s_ctxf = sbt("s_ctxf", (1, 2, NB, ENC_H), FP32)  # ctx rows on partition 0

    estack = ExitStack()

    with tile.TileContext(nc, num_cores=W) as tc:
        pstack = ExitStack()
        pool_sb = pstack.enter_context(tc.tile_pool(name="sb", bufs=3))
        pool_g = pstack.enter_context(tc.tile_pool(name="pg", bufs=2, space="PSUM"))
        pool_pre = pstack.enter_context(tc.tile_pool(name="pp", bufs=2, space="PSUM"))
        pool_sm = pstack.enter_context(tc.tile_pool(name="psm", bufs=2, space="PSUM"))
        pool_tp = pstack.enter_context(tc.tile_pool(name="ptp", bufs=2, space="PSUM"))
        pool_im = pstack.enter_context(tc.tile_pool(name="pim", bufs=2))
        pool_ew = pstack.enter_context(tc.tile_pool(name="pew", bufs=2))

        # ---------------- preamble ----------------
        for dst, src in [
            (s_wax, wih_a_x.transpose([1, 0, 2])),
            (s_wac, wih_a_c.transpose([1, 0, 2])),
            (s_wha, whh_a.transpose([1, 0, 2])),
            (s_wdc, wih_d_c.transpose([1, 0, 2])),
            (s_wdh, wih_d_h.transpose([1, 0, 2])),
            (s_whd, whh_d.transpose([1, 0, 2])),
            (s_ba.ap(), bias_a),
            (s_bd.ap(), bias_d),
            (s_qw, qw_t.transpose([1, 0, 2])),
            (s_mw, mw_t.transpose([1, 0, 2])),
            (s_ww.ap(), ww_t),
            (s_wl.ap(), wl),
            (s_proj, proj_t.transpose([1, 0, 2])),
            (s_pb.ap(), proj_bias),
            (s_prw.ap(), prenet_w16),
            (s_prb.ap(), prenet_b2),
            (s_decT.ap(), dec_inT),
            (s_enc, enc_stat.transpose([2, 0, 1, 3])),
            (s_encT, encT_own.transpose([1, 0, 2])),
            (s_ind.ap(), ind),
        ]:
            d = dst if isinstance(dst, bass.AP) else dst.ap()
            nc.sync.dma_start(d, src)

        make_identity(nc, s_id.ap())
        nc.gpsimd.memset(s_ones.ap(), 1.0)
        nc.gpsimd.memset(s_bufA.ap(), 0.0)
        nc.gpsimd.memset(s_bufB.ap(), 0.0)
        nc.gpsimd.memset(s_payA.ap(), 0.0)
        nc.gpsimd.memset(s_payB.ap(), 0.0)
        nc.gpsimd.memset(s_wbuf.ap(), 0.0)
        nc.gpsimd.memset(s_ca.ap(), 0.0)
        nc.gpsimd.memset(s_cd.ap(), 0.0)
        nc.gpsimd.memset(s_wT.ap(), 0.0)

        # prenet -> xsT
        total = t_dec * B
        if total <= 512:
            NCH = total
        elif total % 500 == 0:
            NCH = 500
        else:
            NCH = 400
        assert total % NCH == 0
        for kt in range(2):
            for c in range((t_dec * B) // NCH):
                ps = pool_pre.tile([128, NCH], FP32, tag="pre")
                nc.tensor.matmul(
                    ps[:, :],
                    s_prw.ap()[:, kt * 128 : (kt + 1) * 128],
                    s_decT.ap()[:, c * NCH : (c + 1) * NCH],
                    start=True,
                    stop=True,
                )
                nc.scalar.activation(
                    s_xsT.ap()[:, kt, c * NCH : (c + 1) * NCH],
                    ps[:, :],
                    AF.Relu,
                    bias=s_prb.ap()[:, kt : kt + 1],
                )

        # pme
        pme_ps = pool_pre.tile([128, LT], FP32, tag="pre")
        for et in range(4):
            nc.tensor.matmul(
                pme_ps[:, :],
                s_mw.ap()[:, et, :],
                s_encT.ap()[:, et, :],
                start=(et == 0),
                stop=(et == 3),
            )
        nc.vector.tensor_copy(s_pme.ap(), pme_ps[:, :])

        # ---------------- helpers ----------------
        def allgather(pay_ap, cin, cout, buf_slice_ap):
            """AllGather pay_ap [128, F] -> buf_slice_ap [128, W, F] via
            DRAM bounce + ncfw collective."""
            nc.sync.dma_start(cin, pay_ap)
            nc.gpsimd.collective_compute(
                "AllGather",
                mybir.AluOpType.bypass,
                replica_groups=[list(range(W))],
                ins=[cin],
                outs=[cout],
            )
            nc.sync.dma_start(buf_slice_ap, cout.transpose([1, 0, 2]))

        def lstm_ew(gates_ps, c_state, hT_dst_ap, tag):
            sig_i = pool_ew.tile([B, HS], FP32, tag=tag + "i")
            sig_f = pool_ew.tile([B, HS], FP32, tag=tag + "f")
            tnh_g = pool_ew.tile([B, HS], FP32, tag=tag + "g")
            sig_o = pool_ew.tile([B, HS], FP32, tag=tag + "o")
            nc.scalar.activation(sig_i[:, :], gates_ps[:, 0:HS], AF.Sigmoid)
            nc.scalar.activation(sig_f[:, :], gates_ps[:, HS : 2 * HS], AF.Sigmoid)
            nc.scalar.activation(tnh_g[:, :], gates_ps[:, 2 * HS : 3 * HS], AF.Tanh)
            nc.scalar.activation(sig_o[:, :], gates_ps[:, 3 * HS : 4 * HS], AF.Sigmoid)
            t1 = pool_ew.tile([B, HS], FP32, tag=tag + "t1")
            t2 = pool_ew.tile([B, HS], FP32, tag=tag + "t2")
            nc.vector.tensor_mul(t1[:, :], sig_f[:, :], c_state.ap())
            nc.vector.tensor_mul(t2[:, :], sig_i[:, :], tnh_g[:, :])
            nc.vector.tensor_add(c_state.ap(), t1[:, :], t2[:, :])
            tnh_c = pool_ew.tile([B, HS], FP32, tag=tag + "tc")
            nc.scalar.activation(tnh_c[:, :], c_state.ap(), AF.Tanh)
            h_new = pool_ew.tile([B, HS], FP32, tag=tag + "h")
            nc.vector.tensor_mul(h_new[:, :], sig_o[:, :], tnh_c[:, :])
            tp = pool_tp.tile([HS, B], FP32, tag="tp")
            nc.tensor.transpose(tp[:, :], h_new[:, :], s_id.ap())
            cp = nc.vector.tensor_copy(hT_dst_ap, tp[:, :])
            return cp

        # ---------------- decode loop ----------------
        pdB = list(s_bufB.ap().ap[0])
        for t in range(t_dec):
            par = t % 3
            parp = (t - 1) % 3  # == 2 for t=0 (zeroed)
            pb = t % 2

            # im2col DMA (reads aw/aws from t-1)
            imc = pool_im.tile([2 * KSIZE, LT], FP32, tag="imc")
            pdim = [list(s_wbuf.ap().ap[0])[0], 1]
            for c in range(2):
                for lb in range(NB):
                    src = bass.AP(
                        s_wbuf,
                        (c * NB + lb) * TEP,
                        [pdim, [1, KSIZE], [1, T_ENC]],
                    )
                    nc.sync.dma_start(
                        imc[
                            c * KSIZE : (c + 1) * KSIZE,
                            lb * T_ENC : (lb + 1) * T_ENC,
                        ],
                        src,
                    )

            # ---- attn-LSTM gates ----
            g_a = pool_g.tile([B, GS], FP32, tag="g")
            nc.tensor.matmul(g_a[:, :], s_ones.ap(), s_ba.ap(), start=True, stop=False)
            for kt in range(2):
                nc.tensor.matmul(
                    g_a[:, :],
                    s_xsT.ap()[:, kt, t * B : (t + 1) * B],
                    s_wax.ap()[:, kt, :],
                    start=False,
                    stop=False,
                )
            ctx_mms = []
            for et in range(4):
                ctx_mms.append(
                    nc.tensor.matmul(
                        g_a[:, :],
                        bass.AP(
                            s_bufB,
                            parp * 64 + et * 16,
                            [pdB, [1, B]],
                        ),
                        s_wac.ap()[:, et, :],
                        start=False,
                        stop=False,
                    )
                )
            ahp_mms = []
            for kt in range(8):
                ahp_mms.append(
                    nc.tensor.matmul(
                        g_a[:, :],
                        s_bufA.ap()[:, parp, kt, 0:B],
                        s_wha.ap()[:, kt, :],
                        start=False,
                        stop=(kt == 7),
                    )
                )

            # ---- attn ew -> payA[:, pb, 0:16], then AG-A ----
            lstm_ew(g_a, s_ca, s_payA.ap()[:, pb, 0:B], "a")
            allgather(
                s_payA.ap()[:, pb, :], cA_in, cA_out, s_bufA.ap()[:, par, :, :]
            )

            # ---- q [16, 128] over full ah(t) ----
            q_ps = pool_sm.tile([B, D], FP32, tag="sm")
            q_mms = []
            for kt in range(8):
                q_mms.append(
                    nc.tensor.matmul(
                        q_ps[:, :],
                        s_bufA.ap()[:, par, kt, 0:B],
                        s_qw.ap()[:, kt, :],
                        start=(kt == 0),
                        stop=(kt == 7),
                    )
                )
            nc.vector.tensor_copy(s_q16.ap()[:, pb, :], q_ps[:, :])

            # ---- attention pre = conv + q_bcast; tanh(pre + pme) ----
            pre_ps = pool_pre.tile([128, LT], FP32, tag="pre")
            nc.tensor.matmul(
                pre_ps[:, :], s_wl.ap(), imc[:, :], start=True, stop=False
            )
            nc.tensor.matmul(
                pre_ps[:, :],
                s_q16.ap()[:, pb, :],
                s_ind.ap(),
                start=False,
                stop=True,
            )
            tmp = pool_sb.tile([128, LT], FP32, tag="tmp")
            nc.vector.tensor_add(tmp[:, :], pre_ps[:, :], s_pme.ap())
            nc.scalar.activation(s_th.ap()[:, pb, :], tmp[:, :], AF.Tanh)

            # ---- energies e [1, 400] ----
            e_ps = pool_sm.tile([1, LT], FP32, tag="sm")
            nc.tensor.matmul(
                e_ps[:, :], s_ww.ap(), s_th.ap()[:, pb, :], start=True, stop=True
            )

            # ---- softmax (no max-sub; energies bounded) ----
            exps = pool_sb.tile([1, LT], FP32, tag="exps")
            ssum = pool_sb.tile([1, NB], FP32, tag="ssum")
            for lb in range(NB):
                nc.scalar.activation(
                    exps[0:1, lb * T_ENC : (lb + 1) * T_ENC],
                    e_ps[0:1, lb * T_ENC : (lb + 1) * T_ENC],
                    AF.Exp,
                    accum_out=ssum[0:1, lb : lb + 1],
                )
            rcp = pool_sb.tile([1, NB], FP32, tag="rcp")
            nc.vector.reciprocal(rcp[:, :], ssum[:, :])
            for lb in range(NB):
                nc.vector.tensor_scalar_mul(
                    s_wbuf.ap()[0:1, 0, lb, 15 : 15 + T_ENC],
                    exps[0:1, lb * T_ENC : (lb + 1) * T_ENC],
                    rcp[0:1, lb : lb + 1],
                )
            # aligns output DMA
            nc.sync.dma_start(
                aligns_out[t], s_wbuf.ap()[0, 0, :, 15 : 15 + T_ENC]
            )
            # aws += aw
            nc.vector.tensor_add(
                s_wbuf.ap()[0:1, 1, :, 15 : 15 + T_ENC],
                s_wbuf.ap()[0:1, 1, :, 15 : 15 + T_ENC],
                s_wbuf.ap()[0:1, 0, :, 15 : 15 + T_ENC],
            )

            # ---- w transposes -> wT [128, (lb, tt)] bf16 ----
            wt_ps = pool_tp.tile([128, NB, 2], FP32, tag="tp")
            for lb in range(NB):
                for tt in range(2):
                    n = 128 if tt == 0 else 72
                    nc.tensor.transpose(
                        wt_ps[0:n, lb, tt : tt + 1],
                        s_wbuf.ap()[0:1, 0, lb, 15 + tt * 128 : 15 + tt * 128 + n],
                        s_id.ap()[0:1, 0:1],
                    )
            nc.vector.tensor_copy(s_wT.ap()[:, pb, :, 0:1], wt_ps[:, :, 0:1])
            nc.vector.tensor_copy(s_wT.ap()[0:72, pb, :, 1:2], wt_ps[0:72, :, 1:2])

            # ---- ctx rows: one [1, 512] psum tile per local batch ----
            for lb in range(NB):
                cx_ps = pool_sm.tile([1, ENC_H], FP32, tag="sm")
                for tt in range(2):
                    nc.tensor.matmul(
                        cx_ps[:, :],
                        s_wT.ap()[:, pb, lb, tt : tt + 1],
                        s_enc.ap()[:, lb, tt, :],
                        start=(tt == 0),
                        stop=(tt == 1),
                    )
                nc.scalar.copy(s_ctxf.ap()[0:1, pb, lb, :], cx_ps[:, :])

            # ---- ctx transposes -> payB [128, (et, lb)] ----
            ct_ps = pool_tp.tile([128, 4, NB], FP32, tag="tp")
            for et in range(4):
                for lb in range(NB):
                    nc.tensor.transpose(
                        ct_ps[:, et, lb : lb + 1],
                        s_ctxf.ap()[0:1, pb, lb, et * 128 : (et + 1) * 128],
                        s_id.ap()[0:1, 0:1],
                    )
            nc.vector.tensor_copy(s_payB.ap()[:, pb, :], ct_ps[:, :, :])
            nc.sync.dma_start(cB_in, s_payB.ap()[:, pb, :])
            nc.gpsimd.collective_compute(
                "AllGather",
                mybir.AluOpType.bypass,
                replica_groups=[list(range(W))],
                ins=[cB_in],
                outs=[cB_out],
            )
            for et in range(4):
                nc.sync.dma_start(
                    bass.AP(s_bufB, par * 64 + et * 16, [pdB, [1, B]]),
                    cB_out[:, :, et * NB : (et + 1) * NB].transpose([1, 0, 2]),
                )

            # ---- dec-LSTM gates ----
            g_d = pool_g.tile([B, GS], FP32, tag="g")
            nc.tensor.matmul(g_d[:, :], s_ones.ap(), s_bd.ap(), start=True, stop=False)
            dc_mms = []
            for et in range(4):
                dc_mms.append(
                    nc.tensor.matmul(
                        g_d[:, :],
                        bass.AP(
                            s_bufB,
                            par * 64 + et * 16,
                            [pdB, [1, B]],
                        ),
                        s_wdc.ap()[:, et, :],
                        start=False,
                        stop=False,
                    )
                )
            for kt in range(8):
                dc_mms.append(
                    nc.tensor.matmul(
                        g_d[:, :],
                        s_bufA.ap()[:, par, kt, 0:B],
                        s_wdh.ap()[:, kt, :],
                        start=False,
                        stop=False,
                    )
                )
            for kt in range(8):
                dc_mms.append(
                    nc.tensor.matmul(
                        g_d[:, :],
                        s_bufA.ap()[:, par, kt, B : 2 * B],
                        s_whd.ap()[:, kt, :],
                        start=False,
                        stop=(kt == 7),
                    )
                )

            # dec ew -> payA[:, (t+1)%2, 16:32]
            lstm_ew(g_d, s_cd, s_payA.ap()[:, (t + 1) % 2, B : 2 * B], "d")

            # ---- proj(t-1) ----
            if t > 0:
                pj_ps = pool_sm.tile([B, N_MELS], FP32, tag="sm")
                nc.tensor.matmul(
                    pj_ps[:, :], s_ones.ap(), s_pb.ap(), start=True, stop=False
                )
                pj_mms = []
                for kt in range(8):
                    pj_mms.append(
                        nc.tensor.matmul(
                            pj_ps[:, :],
                            s_bufA.ap()[:, par, kt, B : 2 * B],
                            s_proj.ap()[:, kt, :],
                            start=False,
                            stop=False,
                        )
                    )
                for et in range(4):
                    pj_mms.append(
                        nc.tensor.matmul(
                            pj_ps[:, :],
                            bass.AP(
                                s_bufB,
                                parp * 64 + et * 16,
                                [pdB, [1, B]],
                            ),
                            s_proj.ap()[:, 8 + et, :],
                            start=False,
                            stop=(et == 3),
                        )
                    )
                mel_sb = pool_sb.tile([B, N_MELS], FP32, tag="mel")
                nc.scalar.copy(mel_sb[:, :], pj_ps[:, :])
                nc.sync.dma_start(mels_out[t - 1], mel_sb[:, :])


        # ---------------- final proj(T-1) ----------------
        t = t_dec
        par = t % 3
        parp = (t - 1) % 3
        allgather(
            s_payA.ap()[:, t % 2, :], cA_in, cA_out, s_bufA.ap()[:, par, :, :]
        )
        pj_ps = pool_sm.tile([B, N_MELS], FP32, tag="sm")
        nc.tensor.matmul(pj_ps[:, :], s_ones.ap(), s_pb.ap(), start=True, stop=False)
        pj_mms = []
        for kt in range(8):
            pj_mms.append(
                nc.tensor.matmul(
                    pj_ps[:, :],
                    s_bufA.ap()[:, par, kt, B : 2 * B],
                    s_proj.ap()[:, kt, :],
                    start=False,
                    stop=False,
                )
            )
        for et in range(4):
            pj_mms.append(
                nc.tensor.matmul(
                    pj_ps[:, :],
                    bass.AP(
                        s_bufB,
                        parp * 64 + et * 16,
                        [pdB, [1, B]],
                    ),
                    s_proj.ap()[:, 8 + et, :],
                    start=False,
                    stop=(et == 3),
                )
            )
        mel_sb = pool_sb.tile([B, N_MELS], FP32, tag="mel")
        nc.scalar.copy(mel_sb[:, :], pj_ps[:, :])
        nc.sync.dma_start(mels_out[t_dec - 1], mel_sb[:, :])
        pstack.close()

    estack.close()
    nc.compile()
    return nc


# ---------------------------------------------------------------------------
# Self-contained harness entry point: kernel(**inputs) -> (mel, align)
# ---------------------------------------------------------------------------
T_DEC_DEFAULT = 250
_cached = {}


def kernel(**inputs):
    from concourse.bass_utils import run_bass_kernel_spmd

    t_dec = int(np.asarray(inputs["targets"]).shape[1])
    if t_dec not in _cached:
        _cached[t_dec] = build_decoder(t_dec)
    nc = _cached[t_dec]
    in_maps = prep_inputs(inputs, t_dec)
    res = run_bass_kernel_spmd(nc, in_maps, core_ids=list(range(W)))
    mel, align = assemble_outputs(res.results, t_dec)
    return mel, align
